# revision 18
# baseline (speedup 1.0000x reference)
"""BPR loss kernel for Trainium2, 8 NeuronCores (SPMD, row-sharded).

Math: with logits = preds[:, :-1, :].reshape(N, V), tgt = targets.reshape(N),
  pos[i] = logits[i, tgt[i]],  neg[i, j] = logits[i, tgt[j]],
  loss = -sum_{i,j valid} log_sigmoid(pos[i] - neg[i, j]) / denom.

The masked double sum is separable over (row i, distinct target v):
  sum_{i,j} m_i m_j ls(pos_i - logits[i, tgt_j])
    = sum_i m_i sum_v c_v ls(pos_i - logits[i, v]),
where c_v = #{j : tgt_j == v, tgt_j != 0}.  Only the U = |{distinct nonzero
targets}| (~3.8K of 32K) columns with c_v > 0 matter, so the host gathers
just those columns (index-derived prep, like the pos gather) and each core
processes its 512-row block of the [N, U] sub-matrix; w = softplus(x - pos)
is reduced over rows with mask weights by PE matmul chains into one f32
PSUM bank-row, and the host finishes the tiny c-weighted dot in f64.

Two independent single-engine element paths (no ACT->DVE chaining, so the
two engines run in parallel on different row blocks):
 * E-path (row-tiles 0-2, on DVE): one fused custom op per tile
     y = x + C0(-pos, per-partition);  u = min(y*y, C1=25);  f = u + C2 u^2
   i.e. softplus(y) ~ y/2 + G0E + LAME*f with a weighted-LSQ deg-2 fit of
   G(s) = ln(2cosh(sqrt(s)/2)) on s in [0,25] (the even part of softplus).
   LAME folds into the PE mask; G0E and the exact linear part y/2 fold into
   the host reduction (sum_i m_i x_iv is a cheap host masked row-sum).
   The clamp at |y|=5 costs ~5e-5 bias; the deg-2 residual is mean-zero
   under the y~N(0,sqrt2) weight and cancels over the 16.8M samples
   (measured end-to-end error ~2.5e-5 vs the 2e-2 gate).
 * A-path (row-tile 3, on ACT): exact softplus = Ln(1 + Exp(y)), two table
   passes, pinned to the natural_log_exp_and_others set (no reloads).

Layout notes (from trace analysis): contiguous per-segment dram tensors
(strided 2D HBM reads are descriptor-gen bound at ~60 GB/s; contiguous
segments reach ~180), aux tensors replicated to >=512B/partition
descriptors, narrow lead E segment (DVE starts right after the first small
DMA lands), narrow tail E segment + cascaded PSUM copies (short tail), and
few, large instructions everywhere (ACT/DVE pay ~1.1-1.4us fixed cost per
instruction; matmuls ~0.25us on top of the 480-column stream).
"""

import numpy as np
import ml_dtypes

import concourse.bass as bass
import concourse.bacc as bacc
import concourse.mybir as mybir
import concourse.tile as tile
from concourse.bass_utils import run_bass_kernel_spmd

# Problem shape (hardcoded; harness contract).
B, L, V = 8, 513, 32000
R = 512            # rows per core
RT = R // 128      # row-tiles per core
FS = 480           # used cols per PSUM bank (bank holds 512 f32)
NBMAX = 8          # PSUM banks per chunk
PADD_IDX = 0
N_CORES = 8
A_TILE = RT - 1    # row-tile on the exact ACT path

# weighted-LSQ deg-2 fit of G(s) = ln(2 cosh(sqrt(s)/2)) on s in [0,25]:
#   G ~ G0E + LAME*(s + QE s^2),  LAME exact in bf16
LAME = 0.10888671875
G0E = 0.7036527
QE = -0.01437426
SCLAMP = 25.0

_f32 = mybir.dt.float32
_bf16 = mybir.dt.bfloat16
_f8 = mybir.dt.float8e3

_compiled = {}

_ACT_SET = "natural_log_exp_and_others"


def _patch_act_tables():
    """Pin Exp and Ln to the one table set containing both, so the greedy
    per-instruction chooser cannot alternate sets and emit ~1.5us
    ACT_TABLE_LOADs mid-kernel.  Set ids are positional, so other sets only
    stop advertising exp/ln; nothing else changes."""
    import concourse.hw_specs as hw_specs
    real = hw_specs.get_activation_tables

    def patched(module_arch):
        t = real(module_arch)
        pin = {
            mybir.ActivationFunctionType.Exp,
            mybir.ActivationFunctionType.Ln,
            mybir.ActivationFunctionType.Copy,
            mybir.ActivationFunctionType.Identity,
        }
        out = {}
        for name, fns in t.items():
            if name != _ACT_SET:
                fns = fns - pin
            out[name] = fns
        return out

    bacc.get_activation_tables = patched


_patch_act_tables()


def _register_dve_op():
    """BPR_EPOLY: y = in0 + C0;  u = min(y*y, C1);  out = u + C2*u*u.
    Single-input custom DVE op (full 1x rate); C0 is a per-partition AP."""
    import concourse.dve_ops as dve_ops
    from concourse.dve_spec import Spec, Src0, C0, C1, C2, minn, lower
    from concourse.dve_spec import _has_src1 as has_src1
    from concourse.dve_uop import DveOpSpec

    for op in dve_ops.OPS:
        if op.name == "BPR_EPOLY":
            return op

    y = Src0 + C0
    u = minn(y * y, C1)
    spec = Spec(
        body=u + C2 * (u * u),
        reference=lambda in0, in1, s0, s1, imm2: (
            lambda u_: u_ + imm2 * u_ * u_
        )(np.minimum(
            (in0.astype(np.float32) + s0) ** 2, s1
        )),
    )
    shas = {}
    for ver in ("v3", "v4"):
        try:
            tmp = DveOpSpec(
                name="BPR_EPOLY", opcode=1, uops=lower(spec, ver=ver),
                rd1_en=has_src1(spec),
            )
            shas[ver] = tmp.sha(ver)
        except Exception:
            pass
    op = dve_ops.DveOp("BPR_EPOLY", spec, subdim=False, uops_sha=shas)
    row = max(dve_ops._SUB_OPCODE_FOR_NAME.values()) + 1
    assert row < 0x20
    dve_ops.OPS.append(op)
    dve_ops._SUB_OPCODE_FOR_NAME["BPR_EPOLY"] = row
    dve_ops.CUSTOM_DVE_SPECS["BPR_EPOLY"] = spec
    return op


EPOLY_OP = _register_dve_op()


def _layout(fc):
    """Per chunk of <=NBMAX*FS cols: E segments (row-tiles 0..RT-2, DVE,
    narrow lead and tail) and A segments (row-tile RT-1 in two column
    halves, ACT).  All column bounds are FS multiples."""
    chunks = []
    c = 0
    while c < fc:
        w = min(fc - c, NBMAX * FS)
        nb = w // FS
        if nb >= 4:
            a = 3 * FS
            b = w - FS
            esegs = [
                (0, 0, a), (0, a, w), (1, 0, w), (2, 0, b), (2, b, w),
            ]
        else:
            esegs = [(r, 0, w) for r in range(RT - 1)]
        asegs = [(0, w)]
        chunks.append((c, w, esegs, asegs))
        c += w
    return chunks


def _build(fc):
    assert fc % FS == 0
    chunks = _layout(fc)
    nc = bacc.Bacc("TRN2", target_bir_lowering=False, debug=False)

    eseg_t = []
    a_t = []
    for ci, (cb, w, esegs, asegs) in enumerate(chunks):
        eseg_t.append([
            nc.dram_tensor(f"xe{ci}_{si}", [128, c1 - c0], _f8,
                           kind="ExternalInput")
            for si, (r, c0, c1) in enumerate(esegs)
        ])
        a_t.append([
            nc.dram_tensor(f"xa{ci}_{ai}", [128, c1 - c0], _f8,
                           kind="ExternalInput")
            for ai, (c0, c1) in enumerate(asegs)
        ])
    np_d = nc.dram_tensor("negpos", [128, 128], _f32, kind="ExternalInput")
    mk_d = nc.dram_tensor("mask", [128, 256], _bf16, kind="ExternalInput")
    nbank_tot = sum((w // FS) for _, w, _, _ in chunks)
    t_d = nc.dram_tensor("t_out", [1, nbank_tot * 512], _f32,
                         kind="ExternalOutput")

    Exp = mybir.ActivationFunctionType.Exp
    Ln = mybir.ActivationFunctionType.Ln

    nes = sum(len(e) for _, _, e, _ in chunks)
    nas = sum(len(a) for _, _, _, a in chunks)
    wmax = max(w for _, w, _, _ in chunks)
    nbmax = max(w // FS for _, w, _, _ in chunks)

    with tile.TileContext(nc) as tc:
        with (
            tc.tile_pool(name="aux", bufs=1) as aux,
            tc.tile_pool(name="xe", bufs=nes) as xepool,
            tc.tile_pool(name="xa", bufs=nas) as xapool,
            tc.tile_pool(name="fp", bufs=nes) as fpool,
            tc.tile_pool(name="wp", bufs=2 * nas) as wpool,
            tc.tile_pool(name="op", bufs=3 * len(chunks)) as opool,
            tc.tile_pool(name="ps", bufs=len(chunks), space="PSUM") as ppool,
        ):
            negpos = aux.tile([128, 128], _f32)
            nc.scalar.dma_start(negpos[:], np_d.ap())
            maskl = aux.tile([128, 256], _bf16)

            def mcol(r):
                return maskl[:, r * 64:r * 64 + 1]

            def npcol(r):
                return negpos[:, r * 32:r * 32 + 1]

            # DMA: per-transfer fixed latency (~1.5-3us) dominates at
            # these sizes, so spread segments over all three rings so the
            # fixed costs overlap: E segments alternate sync/gpsimd
            # (landing in consumption order), aux + A tiles on scalar.
            xetiles = {}
            xatiles = {}
            mk_sent = False
            for ci, (cb, w, esegs, asegs) in enumerate(chunks):
                for si, (r, c0, c1) in enumerate(esegs):
                    xt = xepool.tile([128, wmax], _f8, tag="x")
                    eng = nc.sync if si % 2 == 0 else nc.gpsimd
                    eng.dma_start(xt[:, 0:c1 - c0], eseg_t[ci][si].ap())
                    xetiles[ci, si] = xt
                for ai, (c0, c1) in enumerate(asegs):
                    xt = xapool.tile([128, wmax], _f8, tag="xa")
                    nc.scalar.dma_start(xt[:, 0:c1 - c0], a_t[ci][ai].ap())
                    xatiles[ci, ai] = xt
                    if not mk_sent:
                        nc.scalar.dma_start(maskl[:], mk_d.ap())
                        mk_sent = True
            if not mk_sent:
                nc.scalar.dma_start(maskl[:], mk_d.ap())

            bank_base = 0
            for ci, (cb, w, esegs, asegs) in enumerate(chunks):
                nb = w // FS
                ps = ppool.tile([1, nbmax * 512], _f32, tag="p")

                def mms(ft, foff, r, c0, c1, start, stop):
                    for k in range(c0 // FS, c1 // FS):
                        nc.tensor.matmul(
                            ps[:, k * 512:k * 512 + FS],
                            mcol(r),
                            ft[:, k * FS - foff:(k + 1) * FS - foff],
                            start=start,
                            stop=stop,
                        )

                # A path: exact softplus on ACT (independent of DVE)
                awtiles = []
                for ai, (c0, c1) in enumerate(asegs):
                    wa = c1 - c0
                    et = wpool.tile([128, wmax], _bf16, tag="w")
                    nc.scalar.activation(
                        out=et[:, 0:wa], in_=xatiles[ci, ai][:, 0:wa],
                        func=Exp, bias=npcol(A_TILE), scale=1.0,
                    )
                    wt = wpool.tile([128, wmax], _bf16, tag="w")
                    nc.scalar.activation(
                        out=wt[:, 0:wa], in_=et[:, 0:wa],
                        func=Ln, bias=1.0, scale=1.0,
                    )
                    awtiles.append((wt, c0, c1))

                # E path + PE stream; A matmuls slotted after the last
                # mid-row E segment so the closers stay last.
                close_at = next(
                    i for i, (r, _, _) in enumerate(esegs) if r == RT - 2
                )
                for si, (r, c0, c1) in enumerate(esegs):
                    if si == close_at:
                        for (wt, ca0, ca1) in awtiles:
                            mms(wt, ca0, A_TILE, ca0, ca1, False, False)
                    wseg = c1 - c0
                    ft = fpool.tile([128, wmax], _bf16, tag="f")
                    nc.vector._custom_dve(
                        EPOLY_OP,
                        out=ft[:, 0:wseg],
                        in0=xetiles[ci, si][:, 0:wseg],
                        s0=npcol(r), s1=SCLAMP, imm2=QE,
                    )
                    mms(ft, c0, r, c0, c1, r == 0, r == RT - 2)

                # cascaded copy-out groups; the tail E segment's banks last
                tail0 = esegs[-1][1] // FS if esegs[-1][0] == RT - 2 else 0
                groups = []
                if tail0 > 0:
                    hh = (tail0 + 1) // 2
                    if hh > 0:
                        groups.append((0, hh, "act"))
                    groups.append((hh, tail0, "dve"))
                groups.append((tail0, nb, "act"))
                oeng = [nc.sync, nc.sync, nc.scalar]
                for gi, (k0, k1, eng) in enumerate(groups):
                    ot = opool.tile([1, nbmax * 512], _f32, tag="o")
                    ow = (k1 - k0) * 512
                    if eng == "act":
                        nc.scalar.copy(
                            out=ot[:, 0:ow], in_=ps[:, k0 * 512:k1 * 512]
                        )
                    else:
                        nc.vector.tensor_copy(
                            ot[:, 0:ow], ps[:, k0 * 512:k1 * 512]
                        )
                    oeng[gi % 3].dma_start(
                        t_d.ap()[
                            :,
                            (bank_base + k0) * 512:(bank_base + k1) * 512,
                        ],
                        ot[:, 0:ow],
                    )
                bank_base += nb

    nc.compile()
    return nc, chunks, nbank_tot


def _get_nc(fc):
    if fc not in _compiled:
        _compiled[fc] = _build(fc)
    return _compiled[fc]


def _prep_inputs(preds, targets, chunks):
    """Host-side shard prep: index-derived gathers + exact linear sums."""
    tgt = targets.reshape(-1)
    valid = tgt != PADD_IDX
    n_valid = int(valid.sum())
    u_list = np.unique(tgt[valid])
    U = len(u_list)
    fc = chunks[-1][0] + chunks[-1][1]
    u_pad = np.concatenate(
        [u_list, np.full(fc - U, u_list[0], dtype=u_list.dtype)]
    )

    logits = preds[:, : L - 1, :]
    pos = np.take_along_axis(
        logits, targets[:, :, None], axis=2
    )[:, :, 0]                                          # [B, 512] f32
    maskf = (targets != PADD_IDX).astype(np.float32)

    c = np.bincount(tgt[valid], minlength=V).astype(np.float64)
    c_pad = np.concatenate([c[u_list], np.zeros(fc - U)])
    denom = max(n_valid * n_valid, 1)

    n_e_rows = (RT - 1) * 128       # rows on the E path (rest are A path)
    in_maps = []
    linsums = []
    consts = []
    for d in range(N_CORES):
        X = logits[d][:, u_pad]                         # [512, fc] f32
        m = maskf[d]
        mE = m[:n_e_rows].astype(np.float64)
        linsums.append(mE @ X[:n_e_rows].astype(np.float64))
        Xb = X.astype(ml_dtypes.float8_e3m4)
        npv = (-pos[d]).reshape(RT, 128).T.astype(np.float32)
        mkv = np.empty((128, RT), dtype=np.float32)
        for r in range(RT):
            scale = 1.0 if r == A_TILE else LAME
            mkv[:, r] = scale * m[r * 128:(r + 1) * 128]
        im = {
            "negpos": np.ascontiguousarray(np.repeat(npv, 32, axis=1)),
            "mask": np.ascontiguousarray(
                np.repeat(mkv.astype(ml_dtypes.bfloat16), 64, axis=1)
            ),
        }
        for ci, (cb, w, esegs, asegs) in enumerate(chunks):
            for si, (r, c0, c1) in enumerate(esegs):
                im[f"xe{ci}_{si}"] = np.ascontiguousarray(
                    Xb[r * 128:(r + 1) * 128, cb + c0:cb + c1]
                )
            for ai, (c0, c1) in enumerate(asegs):
                im[f"xa{ci}_{ai}"] = np.ascontiguousarray(
                    Xb[A_TILE * 128:(A_TILE + 1) * 128, cb + c0:cb + c1]
                )
        in_maps.append(im)
        pE = pos[d][:n_e_rows].astype(np.float64)
        consts.append(G0E * mE.sum() - (mE * pE).sum() / 2)
    return in_maps, linsums, consts, c_pad, denom


def _run(preds, targets, trace=False, **spmd_kwargs):
    preds = np.asarray(preds, dtype=np.float32)
    targets_np = np.asarray(targets).astype(np.int64)
    assert preds.shape == (B, L, V), preds.shape
    assert targets_np.shape == (B, L - 1), targets_np.shape

    tgt = targets_np.reshape(-1)
    u_list = np.unique(tgt[tgt != PADD_IDX])
    if len(u_list) == 0:
        return np.array(0.0, dtype=np.float32), None
    fc = ((len(u_list) + FS - 1) // FS) * FS
    nc, chunks, nbank_tot = _get_nc(fc)
    in_maps, linsums, consts, c_pad, denom = _prep_inputs(
        preds, targets_np, chunks
    )
    res = run_bass_kernel_spmd(
        nc, in_maps, core_ids=list(range(N_CORES)), trace=trace, **spmd_kwargs
    )
    csum = float(c_pad.sum())
    loss = 0.0
    for d in range(N_CORES):
        raw = res.results[d]["t_out"].reshape(nbank_tot, 512)[:, :FS]
        t_dev = raw.reshape(-1)[:fc].astype(np.float64)
        loss += float(c_pad @ (t_dev + linsums[d] / 2)) + csum * consts[d]
    loss /= denom
    return np.array(loss, dtype=np.float32), res


def kernel(preds, targets):
    loss, _ = _run(preds, targets, trace=False)
    return loss


# revision 19
# speedup vs baseline: 1.0031x; 1.0031x over previous
"""BPR loss kernel for Trainium2, 8 NeuronCores (SPMD, row-sharded).

Math: with logits = preds[:, :-1, :].reshape(N, V), tgt = targets.reshape(N),
  pos[i] = logits[i, tgt[i]],  neg[i, j] = logits[i, tgt[j]],
  loss = -sum_{i,j valid} log_sigmoid(pos[i] - neg[i, j]) / denom.

The masked double sum is separable over (row i, distinct target v):
  sum_{i,j} m_i m_j ls(pos_i - logits[i, tgt_j])
    = sum_i m_i sum_v c_v ls(pos_i - logits[i, v]),
where c_v = #{j : tgt_j == v, tgt_j != 0}.  Only the U = |{distinct nonzero
targets}| (~3.8K of 32K) columns with c_v > 0 matter, so the host gathers
just those columns (index-derived prep, like the pos gather) and each core
processes its 512-row block of the [N, U] sub-matrix; w = softplus(x - pos)
is reduced over rows with mask weights by PE matmul chains into one f32
PSUM bank-row, and the host finishes the tiny c-weighted dot in f64.

Two independent single-engine element paths (no ACT->DVE chaining, so the
two engines run in parallel on different row blocks):
 * E-path (row-tiles 0-2, on DVE): one fused custom op per tile
     y = x + C0(-pos, per-partition);  u = min(y*y, C1=25);  f = u + C2 u^2
   i.e. softplus(y) ~ y/2 + G0E + LAME*f with a weighted-LSQ deg-2 fit of
   G(s) = ln(2cosh(sqrt(s)/2)) on s in [0,25] (the even part of softplus).
   LAME folds into the PE mask; G0E and the exact linear part y/2 fold into
   the host reduction (sum_i m_i x_iv is a cheap host masked row-sum).
   The clamp at |y|=5 costs ~5e-5 bias; the deg-2 residual is mean-zero
   under the y~N(0,sqrt2) weight and cancels over the 16.8M samples
   (measured end-to-end error ~2.5e-5 vs the 2e-2 gate).
 * A-path (row-tile 3, on ACT): exact softplus = Ln(1 + Exp(y)), two table
   passes, pinned to the natural_log_exp_and_others set (no reloads).

Layout notes (from trace analysis): contiguous per-segment dram tensors
(strided 2D HBM reads are descriptor-gen bound at ~60 GB/s; contiguous
segments reach ~180), aux tensors replicated to >=512B/partition
descriptors, narrow lead E segment (DVE starts right after the first small
DMA lands), narrow tail E segment + cascaded PSUM copies (short tail), and
few, large instructions everywhere (ACT/DVE pay ~1.1-1.4us fixed cost per
instruction; matmuls ~0.25us on top of the 480-column stream).
"""

import numpy as np
import ml_dtypes

import concourse.bass as bass
import concourse.bacc as bacc
import concourse.mybir as mybir
import concourse.tile as tile
from concourse.bass_utils import run_bass_kernel_spmd

# Problem shape (hardcoded; harness contract).
B, L, V = 8, 513, 32000
R = 512            # rows per core
RT = R // 128      # row-tiles per core
FS = 480           # used cols per PSUM bank (bank holds 512 f32)
NBMAX = 8          # PSUM banks per chunk
PADD_IDX = 0
N_CORES = 8
A_TILE = RT - 1    # row-tile on the exact ACT path

# weighted-LSQ deg-2 fit of G(s) = ln(2 cosh(sqrt(s)/2)) on s in [0,25]:
#   G ~ G0E + LAME*(s + QE s^2),  LAME exact in bf16
LAME = 0.10888671875
G0E = 0.7036527
QE = -0.01437426
SCLAMP = 25.0

_f32 = mybir.dt.float32
_bf16 = mybir.dt.bfloat16
_f8 = mybir.dt.float8e3

_compiled = {}

_ACT_SET = "natural_log_exp_and_others"


def _patch_act_tables():
    """Pin Exp and Ln to the one table set containing both, so the greedy
    per-instruction chooser cannot alternate sets and emit ~1.5us
    ACT_TABLE_LOADs mid-kernel.  Set ids are positional, so other sets only
    stop advertising exp/ln; nothing else changes."""
    import concourse.hw_specs as hw_specs
    real = hw_specs.get_activation_tables

    def patched(module_arch):
        t = real(module_arch)
        pin = {
            mybir.ActivationFunctionType.Exp,
            mybir.ActivationFunctionType.Ln,
            mybir.ActivationFunctionType.Copy,
            mybir.ActivationFunctionType.Identity,
        }
        out = {}
        for name, fns in t.items():
            if name != _ACT_SET:
                fns = fns - pin
            out[name] = fns
        return out

    bacc.get_activation_tables = patched


_patch_act_tables()


def _register_dve_op():
    """BPR_EPOLY: y = in0 + C0;  u = min(y*y, C1);  out = u + C2*u*u.
    Single-input custom DVE op (full 1x rate); C0 is a per-partition AP."""
    import concourse.dve_ops as dve_ops
    from concourse.dve_spec import Spec, Src0, C0, C1, C2, minn, lower
    from concourse.dve_spec import _has_src1 as has_src1
    from concourse.dve_uop import DveOpSpec

    for op in dve_ops.OPS:
        if op.name == "BPR_EPOLY":
            return op

    y = Src0 + C0
    u = minn(y * y, C1)
    spec = Spec(
        body=u + C2 * (u * u),
        reference=lambda in0, in1, s0, s1, imm2: (
            lambda u_: u_ + imm2 * u_ * u_
        )(np.minimum(
            (in0.astype(np.float32) + s0) ** 2, s1
        )),
    )
    shas = {}
    for ver in ("v3", "v4"):
        try:
            tmp = DveOpSpec(
                name="BPR_EPOLY", opcode=1, uops=lower(spec, ver=ver),
                rd1_en=has_src1(spec),
            )
            shas[ver] = tmp.sha(ver)
        except Exception:
            pass
    op = dve_ops.DveOp("BPR_EPOLY", spec, subdim=False, uops_sha=shas)
    row = max(dve_ops._SUB_OPCODE_FOR_NAME.values()) + 1
    assert row < 0x20
    dve_ops.OPS.append(op)
    dve_ops._SUB_OPCODE_FOR_NAME["BPR_EPOLY"] = row
    dve_ops.CUSTOM_DVE_SPECS["BPR_EPOLY"] = spec
    return op


EPOLY_OP = _register_dve_op()


def _layout(fc):
    """Per chunk of <=NBMAX*FS cols: E segments (row-tiles 0..RT-2, DVE,
    narrow lead and tail) and A segments (row-tile RT-1 in two column
    halves, ACT).  All column bounds are FS multiples."""
    chunks = []
    c = 0
    while c < fc:
        w = min(fc - c, NBMAX * FS)
        nb = w // FS
        if nb >= 4:
            a = 3 * FS
            b = w - 2 * FS
            esegs = [
                (0, 0, a), (0, a, w), (1, 0, w), (2, 0, b), (2, b, w),
            ]
        else:
            esegs = [(r, 0, w) for r in range(RT - 1)]
        asegs = [(0, w)]
        chunks.append((c, w, esegs, asegs))
        c += w
    return chunks


def _build(fc):
    assert fc % FS == 0
    chunks = _layout(fc)
    nc = bacc.Bacc("TRN2", target_bir_lowering=False, debug=False)

    eseg_t = []
    a_t = []
    for ci, (cb, w, esegs, asegs) in enumerate(chunks):
        eseg_t.append([
            nc.dram_tensor(f"xe{ci}_{si}", [128, c1 - c0], _f8,
                           kind="ExternalInput")
            for si, (r, c0, c1) in enumerate(esegs)
        ])
        a_t.append([
            nc.dram_tensor(f"xa{ci}_{ai}", [128, c1 - c0], _f8,
                           kind="ExternalInput")
            for ai, (c0, c1) in enumerate(asegs)
        ])
    np_d = nc.dram_tensor("negpos", [128, 128], _f32, kind="ExternalInput")
    mk_d = nc.dram_tensor("mask", [128, 256], _bf16, kind="ExternalInput")
    nbank_tot = sum((w // FS) for _, w, _, _ in chunks)
    t_d = nc.dram_tensor("t_out", [1, nbank_tot * 512], _f32,
                         kind="ExternalOutput")

    Exp = mybir.ActivationFunctionType.Exp
    Ln = mybir.ActivationFunctionType.Ln

    nes = sum(len(e) for _, _, e, _ in chunks)
    nas = sum(len(a) for _, _, _, a in chunks)
    wmax = max(w for _, w, _, _ in chunks)
    nbmax = max(w // FS for _, w, _, _ in chunks)

    with tile.TileContext(nc) as tc:
        with (
            tc.tile_pool(name="aux", bufs=1) as aux,
            tc.tile_pool(name="xe", bufs=nes) as xepool,
            tc.tile_pool(name="xa", bufs=nas) as xapool,
            tc.tile_pool(name="fp", bufs=nes) as fpool,
            tc.tile_pool(name="wp", bufs=2 * nas) as wpool,
            tc.tile_pool(name="op", bufs=3 * len(chunks)) as opool,
            tc.tile_pool(name="ps", bufs=len(chunks), space="PSUM") as ppool,
        ):
            negpos = aux.tile([128, 128], _f32)
            nc.scalar.dma_start(negpos[:], np_d.ap())
            maskl = aux.tile([128, 256], _bf16)

            def mcol(r):
                return maskl[:, r * 64:r * 64 + 1]

            def npcol(r):
                return negpos[:, r * 32:r * 32 + 1]

            # DMA: per-transfer fixed latency (~1.5-3us) dominates at
            # these sizes, so spread segments over all three rings so the
            # fixed costs overlap: E segments alternate sync/gpsimd
            # (landing in consumption order), aux + A tiles on scalar.
            xetiles = {}
            xatiles = {}
            mk_sent = False
            for ci, (cb, w, esegs, asegs) in enumerate(chunks):
                for si, (r, c0, c1) in enumerate(esegs):
                    xt = xepool.tile([128, wmax], _f8, tag="x")
                    eng = nc.sync if si % 2 == 0 else nc.gpsimd
                    eng.dma_start(xt[:, 0:c1 - c0], eseg_t[ci][si].ap())
                    xetiles[ci, si] = xt
                for ai, (c0, c1) in enumerate(asegs):
                    xt = xapool.tile([128, wmax], _f8, tag="xa")
                    nc.scalar.dma_start(xt[:, 0:c1 - c0], a_t[ci][ai].ap())
                    xatiles[ci, ai] = xt
                    if not mk_sent:
                        nc.scalar.dma_start(maskl[:], mk_d.ap())
                        mk_sent = True
            if not mk_sent:
                nc.scalar.dma_start(maskl[:], mk_d.ap())

            bank_base = 0
            for ci, (cb, w, esegs, asegs) in enumerate(chunks):
                nb = w // FS
                ps = ppool.tile([1, nbmax * 512], _f32, tag="p")

                def mms(ft, foff, r, c0, c1, start, stop):
                    for k in range(c0 // FS, c1 // FS):
                        nc.tensor.matmul(
                            ps[:, k * 512:k * 512 + FS],
                            mcol(r),
                            ft[:, k * FS - foff:(k + 1) * FS - foff],
                            start=start,
                            stop=stop,
                        )

                # A path: exact softplus on ACT (independent of DVE)
                awtiles = []
                for ai, (c0, c1) in enumerate(asegs):
                    wa = c1 - c0
                    et = wpool.tile([128, wmax], _bf16, tag="w")
                    nc.scalar.activation(
                        out=et[:, 0:wa], in_=xatiles[ci, ai][:, 0:wa],
                        func=Exp, bias=npcol(A_TILE), scale=1.0,
                    )
                    wt = wpool.tile([128, wmax], _bf16, tag="w")
                    nc.scalar.activation(
                        out=wt[:, 0:wa], in_=et[:, 0:wa],
                        func=Ln, bias=1.0, scale=1.0,
                    )
                    awtiles.append((wt, c0, c1))

                # E path + PE stream; A matmuls slotted after the last
                # mid-row E segment so the closers stay last.
                close_at = next(
                    i for i, (r, _, _) in enumerate(esegs) if r == RT - 2
                )
                for si, (r, c0, c1) in enumerate(esegs):
                    if si == close_at:
                        for (wt, ca0, ca1) in awtiles:
                            mms(wt, ca0, A_TILE, ca0, ca1, False, False)
                    wseg = c1 - c0
                    ft = fpool.tile([128, wmax], _bf16, tag="f")
                    nc.vector._custom_dve(
                        EPOLY_OP,
                        out=ft[:, 0:wseg],
                        in0=xetiles[ci, si][:, 0:wseg],
                        s0=npcol(r), s1=SCLAMP, imm2=QE,
                    )
                    mms(ft, c0, r, c0, c1, r == 0, r == RT - 2)

                # cascaded copy-out groups; the tail E segment's banks last
                tail0 = esegs[-1][1] // FS if esegs[-1][0] == RT - 2 else 0
                groups = []
                if tail0 > 0:
                    hh = tail0 // 2
                    if hh > 0:
                        groups.append((0, hh, "act"))
                    groups.append((hh, tail0, "dve"))
                groups.append((tail0, nb, "act"))
                oeng = [nc.sync, nc.sync, nc.scalar]
                for gi, (k0, k1, eng) in enumerate(groups):
                    ot = opool.tile([1, nbmax * 512], _f32, tag="o")
                    ow = (k1 - k0) * 512
                    if eng == "act":
                        nc.scalar.copy(
                            out=ot[:, 0:ow], in_=ps[:, k0 * 512:k1 * 512]
                        )
                    else:
                        nc.vector.tensor_copy(
                            ot[:, 0:ow], ps[:, k0 * 512:k1 * 512]
                        )
                    oeng[gi % 3].dma_start(
                        t_d.ap()[
                            :,
                            (bank_base + k0) * 512:(bank_base + k1) * 512,
                        ],
                        ot[:, 0:ow],
                    )
                bank_base += nb

    nc.compile()
    return nc, chunks, nbank_tot


def _get_nc(fc):
    if fc not in _compiled:
        _compiled[fc] = _build(fc)
    return _compiled[fc]


def _prep_inputs(preds, targets, chunks):
    """Host-side shard prep: index-derived gathers + exact linear sums."""
    tgt = targets.reshape(-1)
    valid = tgt != PADD_IDX
    n_valid = int(valid.sum())
    u_list = np.unique(tgt[valid])
    U = len(u_list)
    fc = chunks[-1][0] + chunks[-1][1]
    u_pad = np.concatenate(
        [u_list, np.full(fc - U, u_list[0], dtype=u_list.dtype)]
    )

    logits = preds[:, : L - 1, :]
    pos = np.take_along_axis(
        logits, targets[:, :, None], axis=2
    )[:, :, 0]                                          # [B, 512] f32
    maskf = (targets != PADD_IDX).astype(np.float32)

    c = np.bincount(tgt[valid], minlength=V).astype(np.float64)
    c_pad = np.concatenate([c[u_list], np.zeros(fc - U)])
    denom = max(n_valid * n_valid, 1)

    n_e_rows = (RT - 1) * 128       # rows on the E path (rest are A path)
    in_maps = []
    linsums = []
    consts = []
    for d in range(N_CORES):
        X = logits[d][:, u_pad]                         # [512, fc] f32
        m = maskf[d]
        mE = m[:n_e_rows].astype(np.float64)
        linsums.append(mE @ X[:n_e_rows].astype(np.float64))
        Xb = X.astype(ml_dtypes.float8_e3m4)
        npv = (-pos[d]).reshape(RT, 128).T.astype(np.float32)
        mkv = np.empty((128, RT), dtype=np.float32)
        for r in range(RT):
            scale = 1.0 if r == A_TILE else LAME
            mkv[:, r] = scale * m[r * 128:(r + 1) * 128]
        im = {
            "negpos": np.ascontiguousarray(np.repeat(npv, 32, axis=1)),
            "mask": np.ascontiguousarray(
                np.repeat(mkv.astype(ml_dtypes.bfloat16), 64, axis=1)
            ),
        }
        for ci, (cb, w, esegs, asegs) in enumerate(chunks):
            for si, (r, c0, c1) in enumerate(esegs):
                im[f"xe{ci}_{si}"] = np.ascontiguousarray(
                    Xb[r * 128:(r + 1) * 128, cb + c0:cb + c1]
                )
            for ai, (c0, c1) in enumerate(asegs):
                im[f"xa{ci}_{ai}"] = np.ascontiguousarray(
                    Xb[A_TILE * 128:(A_TILE + 1) * 128, cb + c0:cb + c1]
                )
        in_maps.append(im)
        pE = pos[d][:n_e_rows].astype(np.float64)
        consts.append(G0E * mE.sum() - (mE * pE).sum() / 2)
    return in_maps, linsums, consts, c_pad, denom


def _run(preds, targets, trace=False, **spmd_kwargs):
    preds = np.asarray(preds, dtype=np.float32)
    targets_np = np.asarray(targets).astype(np.int64)
    assert preds.shape == (B, L, V), preds.shape
    assert targets_np.shape == (B, L - 1), targets_np.shape

    tgt = targets_np.reshape(-1)
    u_list = np.unique(tgt[tgt != PADD_IDX])
    if len(u_list) == 0:
        return np.array(0.0, dtype=np.float32), None
    fc = ((len(u_list) + FS - 1) // FS) * FS
    nc, chunks, nbank_tot = _get_nc(fc)
    in_maps, linsums, consts, c_pad, denom = _prep_inputs(
        preds, targets_np, chunks
    )
    res = run_bass_kernel_spmd(
        nc, in_maps, core_ids=list(range(N_CORES)), trace=trace, **spmd_kwargs
    )
    csum = float(c_pad.sum())
    loss = 0.0
    for d in range(N_CORES):
        raw = res.results[d]["t_out"].reshape(nbank_tot, 512)[:, :FS]
        t_dev = raw.reshape(-1)[:fc].astype(np.float64)
        loss += float(c_pad @ (t_dev + linsums[d] / 2)) + csum * consts[d]
    loss /= denom
    return np.array(loss, dtype=np.float32), res


def kernel(preds, targets):
    loss, _ = _run(preds, targets, trace=False)
    return loss
